# revision 1
# baseline (speedup 1.0000x reference)
"""RGCN (5 relations, 3 RGCN layers + mean readout + MLP head) on 8 trn2 cores.

Sharding: data-parallel over destination-node slices (12500/core). Per layer,
mean-aggregation of raw inputs per relation is done with one-hot matmuls on
the TensorEngine (aggregate-then-multiply), followed by one dense [f,h] matmul
per relation; h is kept feature-major (bf16) in SBUF. Cross-layer exchange is
an AllGather of row-major h. Gathers use dma_gather via per-relation compacted
tables (16384 rows, int16-safe) built on device. Readout via a host-built
selection-matrix matmul + AllReduce; the small MLP head is replicated.
"""

import sys
import numpy as np

sys.path.insert(0, "/opt/trn_rl_repo")

import ml_dtypes  # noqa: E402

BF16 = ml_dtypes.bfloat16

N = 100000
G = 256
E = 120000
IN = 162
HID = 128
R = 5
L = 2
NC = 8
SLICE = N // NC            # 12500
NW = (SLICE + 127) // 128  # 98 windows (last partial: 84 nodes)
TPW = 2                    # tiles per window (256-edge capacity per (r,w))
NTILES = NW * TPW          # 196
SLOTS = NTILES * 128       # 25088
QROWS = N // 4             # 25000
CCAP = 4096                # compact rows per quarter -> table 16384 rows
TROWS = 4 * CCAP
NCH = 7                    # stream chunks per relation
WPC = 7                    # window-pairs per chunk (NCH*WPC*2 == NW)
TPC = WPC * 2 * TPW        # 28 tiles per chunk
SPC = TPC * 128            # 3584 slots per chunk


def _wrap_idx(idx):
    n = len(idx)
    w = np.zeros((128, n // 16), np.int16)
    m = idx.reshape(n // 16, 16).T
    for k in range(8):
        w[16 * k:16 * (k + 1), :] = m
    return w


def _prep_core(c, X, srcs, dsts, batch_np, cnts):
    lo, hi = c * SLICE, (c + 1) * SLICE
    l0pack = np.zeros((R, 128, NTILES * IN), BF16)
    onehot = np.zeros((R, 128, NTILES * 128), BF16)
    eidx = np.zeros((R, 128, SLOTS // 16), np.int16)
    cidx = np.zeros((R, 4, 128, CCAP // 16), np.int16)
    for r in range(R):
        m = (dsts[r] >= lo) & (dsts[r] < hi)
        s = srcs[r][m]
        dg = dsts[r][m]
        o = np.argsort(dg - lo, kind="stable")
        s, dg = s[o], dg[o]
        d = dg - lo
        u = np.unique(s)
        uq = [u[(u >= q * QROWS) & (u < (q + 1) * QROWS)] for q in range(4)]
        for q in range(4):
            k = len(uq[q])
            assert k <= CCAP, (c, r, q, k)
            ci = np.zeros(CCAP, np.int16)
            ci[:k] = (uq[q] - q * QROWS).astype(np.int16)
            cidx[r, q] = _wrap_idx(ci)
        base = np.array([0, len(uq[0]), len(uq[0]) + len(uq[1]),
                         len(uq[0]) + len(uq[1]) + len(uq[2])])
        cat = np.concatenate(uq)
        qo = s // QROWS
        pos = (qo * CCAP + (np.searchsorted(cat, s) - base[qo])).astype(np.int64)
        w_of = d // 128
        wc = np.bincount(w_of, minlength=NW)
        assert wc.max() <= TPW * 128, (c, r, wc.max())
        start = np.concatenate([[0], np.cumsum(wc)[:-1]])
        slot = w_of * (TPW * 128) + (np.arange(len(d)) - start[w_of])
        e_arr = np.zeros(SLOTS, np.int64)
        e_arr[slot] = pos
        eidx[r] = _wrap_idx(e_arr.astype(np.int16))
        lane = slot % 128
        tcol = slot // 128
        colw = d - 128 * w_of
        vals = (1.0 / cnts[r][dg]).astype(np.float32)
        oh = np.zeros((128, NTILES, 128), np.float32)
        oh[lane, tcol, colw] = vals
        onehot[r] = oh.reshape(128, NTILES * 128).astype(BF16)
        lp = np.zeros((128, NTILES, IN), np.float32)
        lp[lane, tcol, :] = X[s]
        l0pack[r] = lp.reshape(128, NTILES * IN).astype(BF16)
    xmyt = np.ascontiguousarray(X[lo:hi].T).astype(BF16)
    sel = np.zeros((128, NW, G), np.float32)
    bat = batch_np[lo:hi]
    node = np.arange(SLICE)
    gcnt = np.maximum(np.bincount(batch_np, minlength=G), 1).astype(np.float32)
    sel[node % 128, node // 128, bat] = 1.0 / gcnt[bat]
    sel = sel.reshape(128, NW * G)
    return dict(l0pack=l0pack, onehot=onehot, eidx=eidx, cidx=cidx,
                xmyt=xmyt, sel=sel.astype(BF16))


def kernel(X, edge_index1, edge_index2, edge_index3, edge_index4, edge_index5,
           batch, W0, root0, b0, Wl, rootl, bl, Wc1, bc1, Wc2, bc2, Wc3, bc3):
    import concourse.bass as bass  # noqa: F401
    import concourse.bacc as bacc
    import concourse.mybir as mybir
    import concourse.tile as tile
    from concourse.bass_utils import run_bass_kernel_spmd
    from concourse.masks import make_identity

    X = np.asarray(X, np.float32)
    batch_np = np.asarray(batch).astype(np.int64)
    eis = [np.asarray(e).astype(np.int64) for e in
           (edge_index1, edge_index2, edge_index3, edge_index4, edge_index5)]
    srcs = [e[0] for e in eis]
    dsts = [e[1] for e in eis]
    cnts = [np.maximum(np.bincount(dsts[r], minlength=N), 1).astype(np.float32)
            for r in range(R)]
    per_core = [_prep_core(c, X, srcs, dsts, batch_np, cnts) for c in range(NC)]

    f32, bf16, i16 = mybir.dt.float32, mybir.dt.bfloat16, mybir.dt.int16
    f16 = mybir.dt.float16
    f8 = mybir.dt.float8e4

    nc = bacc.Bacc("TRN2", target_bir_lowering=False, debug=False)
    l0pack_d = nc.declare_dram_parameter("l0pack", [R, 128, NTILES * IN], bf16, isOutput=False)
    onehot_d = nc.declare_dram_parameter("onehot", [R, 128, NTILES * 128], bf16, isOutput=False)
    eidx_d = nc.declare_dram_parameter("eidx", [R, 128, SLOTS // 16], i16, isOutput=False)
    cidx_d = nc.declare_dram_parameter("cidx", [R, 4, 128, CCAP // 16], i16, isOutput=False)
    xmyt_d = nc.declare_dram_parameter("xmyt", [IN, SLICE], bf16, isOutput=False)
    sel_d = nc.declare_dram_parameter("sel", [128, NW * G], bf16, isOutput=False)
    w0hi_d = nc.declare_dram_parameter("w0hi", [128, R * HID], bf16, isOutput=False)
    w0lo_d = nc.declare_dram_parameter("w0lo", [IN - 128, R * HID], bf16, isOutput=False)
    wl_d = nc.declare_dram_parameter("wl", [HID, L * R * HID], bf16, isOutput=False)
    root0_d = nc.declare_dram_parameter("root0", [IN, HID], bf16, isOutput=False)
    rootl_d = nc.declare_dram_parameter("rootl", [HID, L * HID], bf16, isOutput=False)
    b0_d = nc.declare_dram_parameter("b0", [HID, 1], f32, isOutput=False)
    bl_d = nc.declare_dram_parameter("bl", [HID, L], f32, isOutput=False)
    wc1_d = nc.declare_dram_parameter("wc1", [HID, HID], bf16, isOutput=False)
    wc2_d = nc.declare_dram_parameter("wc2", [HID, HID], bf16, isOutput=False)
    wc3_d = nc.declare_dram_parameter("wc3", [HID, 1], bf16, isOutput=False)
    bc1_d = nc.declare_dram_parameter("bc1", [HID, 1], f32, isOutput=False)
    bc2_d = nc.declare_dram_parameter("bc2", [HID, 1], f32, isOutput=False)
    bc3_d = nc.declare_dram_parameter("bc3", [1, 1], f32, isOutput=False)
    out_d = nc.declare_dram_parameter("out", [1, G], f32, isOutput=True)

    hrows = nc.dram_tensor("hrows", [SLICE, HID], bf16)
    hfull = nc.dram_tensor("hfull", [N, HID], bf16, addr_space="Shared")
    trs = [nc.dram_tensor(f"tr{r}", [TROWS, HID], bf16) for r in range(R)]
    ar_in = nc.dram_tensor("ar_in", [HID, G], f32)
    ar_out = nc.dram_tensor("ar_out", [HID, G], f32, addr_space="Shared")

    WPAIR = NW // 2

    with tile.TileContext(nc) as tc:
        with tc.tile_pool(name="const", bufs=1) as cpool, \
             tc.tile_pool(name="hbuf", bufs=1) as hpool, \
             tc.tile_pool(name="work", bufs=3) as wpool, \
             tc.tile_pool(name="stg", bufs=2) as spool, \
             tc.tile_pool(name="edge", bufs=3) as epool, \
             tc.tile_pool(name="oh", bufs=3) as opool, \
             tc.tile_pool(name="ps", bufs=2, space="PSUM") as pp:

            ident = cpool.tile([128, 128], bf16, tag="ident")
            make_identity(nc, ident[:])
            w0hi_t = cpool.tile([128, R * HID], bf16, tag="w0hi")
            nc.sync.dma_start(out=w0hi_t[:], in_=w0hi_d[:])
            w0lo_t = cpool.tile([IN - 128, R * HID], bf16, tag="w0lo")
            nc.sync.dma_start(out=w0lo_t[:], in_=w0lo_d[:])
            wl_t = cpool.tile([HID, L * R * HID], bf16, tag="wlt")
            nc.sync.dma_start(out=wl_t[:], in_=wl_d[:])
            root0hi_t = cpool.tile([128, HID], bf16, tag="root0hi")
            nc.sync.dma_start(out=root0hi_t[:], in_=root0_d[0:128, :])
            root0lo_t = cpool.tile([IN - 128, HID], bf16, tag="root0lo")
            nc.sync.dma_start(out=root0lo_t[:], in_=root0_d[128:IN, :])
            rootl_t = cpool.tile([HID, L * HID], bf16, tag="rootlt")
            nc.sync.dma_start(out=rootl_t[:], in_=rootl_d[:])
            b0_t = cpool.tile([HID, 1], f32, tag="b0t")
            nc.sync.dma_start(out=b0_t[:], in_=b0_d[:])
            bl_t = cpool.tile([HID, L], f32, tag="blt")
            nc.sync.dma_start(out=bl_t[:], in_=bl_d[:])

            h_cur = hpool.tile([128, SLICE], bf16, tag="hcur")
            h_acc = hpool.tile([128, SLICE], f16, tag="hacc")

            def layer_body(layer):
                fstep = IN if layer == 0 else HID
                # --- init h_acc with root contribution (reads old h_cur) ---
                for wp in range(WPAIR):
                    cs = wp * 256
                    cl = min(256, SLICE - cs)
                    ps = pp.tile([128, 256], f32, space="PSUM", tag="d")
                    if layer == 0:
                        xh = wpool.tile([128, 256], bf16, tag="xsl")
                        nc.sync.dma_start(out=xh[:, :cl], in_=xmyt_d[0:128, cs:cs + cl])
                        xl = wpool.tile([IN - 128, 256], bf16, tag="xsl2")
                        nc.sync.dma_start(out=xl[:, :cl], in_=xmyt_d[128:IN, cs:cs + cl])
                        nc.tensor.matmul(ps[:, :cl], root0hi_t[:], xh[:, :cl], start=True, stop=False)
                        nc.tensor.matmul(ps[:, :cl], root0lo_t[:], xl[:, :cl], start=False, stop=True)
                    else:
                        nc.tensor.matmul(ps[:, :cl], rootl_t[:, (layer - 1) * HID:layer * HID], h_cur[:, cs:cs + cl], start=True, stop=True)
                    nc.scalar.activation(out=h_acc[:, cs:cs + cl], in_=ps[:, :cl],
                                         func=mybir.ActivationFunctionType.Copy)
                for r in range(R):
                    wmat = None if layer == 0 else wl_t[:, ((layer - 1) * R + r) * HID:((layer - 1) * R + r + 1) * HID]
                    if layer != 0:
                        for q in range(4):
                            ci = wpool.tile([128, CCAP // 16], i16, tag="ci")
                            nc.sync.dma_start(out=ci[:], in_=cidx_d[r, q])
                            st = spool.tile([128, (CCAP // 128) * HID], bf16, tag="stage")
                            nc.gpsimd.dma_gather(
                                out_ap=st[:].rearrange("p (t f) -> p t f", f=HID),
                                in_ap=hfull[q * QROWS:(q + 1) * QROWS, :],
                                idxs_ap=ci[:],
                                num_idxs=CCAP, num_idxs_reg=CCAP,
                                elem_size=HID, single_packet=False)
                            nc.sync.dma_start(
                                out=trs[r][q * CCAP:(q + 1) * CCAP, :].rearrange(
                                    "(t p) f -> p t f", p=128),
                                in_=st[:].rearrange("p (t f) -> p t f", f=HID))
                        ei = spool.tile([128, SLOTS // 16], i16, tag="ei")
                        nc.sync.dma_start(out=ei[:], in_=eidx_d[r])
                    for ch in range(NCH):
                        t0 = ch * TPC
                        ebuf = epool.tile([128, TPC * IN], bf16, tag="ebuf")
                        if layer == 0:
                            nc.sync.dma_start(
                                out=ebuf[:, :TPC * IN],
                                in_=l0pack_d[r][:, t0 * IN:(t0 + TPC) * IN])
                        else:
                            nc.gpsimd.dma_gather(
                                out_ap=ebuf[:, :TPC * HID].rearrange("p (t f) -> p t f", f=HID),
                                in_ap=trs[r][:],
                                idxs_ap=ei[:, (t0 * 128) // 16:((t0 + TPC) * 128) // 16],
                                num_idxs=SPC, num_idxs_reg=SPC,
                                elem_size=HID, single_packet=False)
                        ohb = opool.tile([128, TPC * 128], bf16, tag="ohb")
                        nc.sync.dma_start(
                            out=ohb[:], in_=onehot_d[r][:, t0 * 128:(t0 + TPC) * 128])
                        for wpl in range(WPC):
                            wp = ch * WPC + wpl
                            aps = pp.tile([128, 256], f32, space="PSUM", tag="a")
                            if layer == 0:
                                aps2 = pp.tile([IN - 128, 256], f32, space="PSUM", tag="a2")
                            for half in range(2):
                                for t in range(TPW):
                                    ti = (wpl * 2 + half) * TPW + t
                                    et = ebuf[:, ti * fstep:ti * fstep + fstep]
                                    oh = ohb[:, ti * 128:(ti + 1) * 128]
                                    st0, sp0 = (t == 0), (t == TPW - 1)
                                    nc.tensor.matmul(
                                        aps[:, half * 128:(half + 1) * 128],
                                        et[:, 0:128], oh, start=st0, stop=sp0)
                                    if layer == 0:
                                        nc.tensor.matmul(
                                            aps2[:, half * 128:(half + 1) * 128],
                                            et[:, 128:IN], oh, start=st0, stop=sp0)
                            a_sb = wpool.tile([128, 256], bf16, tag="asb")
                            nc.scalar.activation(out=a_sb[:], in_=aps[:],
                                                 func=mybir.ActivationFunctionType.Copy)
                            dps = pp.tile([128, 256], f32, space="PSUM", tag="d")
                            if layer == 0:
                                a_sb2 = wpool.tile([IN - 128, 256], bf16, tag="asb2")
                                nc.scalar.activation(out=a_sb2[:], in_=aps2[:],
                                                     func=mybir.ActivationFunctionType.Copy)
                                nc.tensor.matmul(dps[:], w0hi_t[:, r * HID:(r + 1) * HID], a_sb[:], start=True, stop=False)
                                nc.tensor.matmul(dps[:], w0lo_t[:, r * HID:(r + 1) * HID], a_sb2[:], start=False, stop=True)
                            else:
                                nc.tensor.matmul(dps[:], wmat, a_sb[:], start=True, stop=True)
                            cs = wp * 256
                            cl = min(256, SLICE - cs)
                            nc.vector.tensor_tensor(
                                out=h_acc[:, cs:cs + cl], in0=dps[:, :cl],
                                in1=h_acc[:, cs:cs + cl], op=mybir.AluOpType.add)
                bias = b0_t[:] if layer == 0 else bl_t[:, layer - 1:layer]
                for wp in range(WPAIR):
                    cs = wp * 256
                    cl = min(256, SLICE - cs)
                    nc.scalar.activation(
                        out=h_cur[:, cs:cs + cl], in_=h_acc[:, cs:cs + cl],
                        func=mybir.ActivationFunctionType.Relu,
                        bias=bias, scale=1.0)

            def transpose_rows(write_rows, mm_readout, selb):
                rps = None
                if mm_readout:
                    rps = pp.tile([128, G], f32, space="PSUM", tag="a2")
                for w in range(NW):
                    cs = w * 128
                    cl = min(128, SLICE - cs)
                    tp = pp.tile([128, 128], bf16, space="PSUM", tag="a")
                    nc.tensor.transpose(out=tp[:cl, :], in_=h_cur[:, cs:cs + cl], identity=ident[:])
                    rt = wpool.tile([128, 128], bf16, tag="rowt")
                    nc.scalar.activation(out=rt[:cl, :], in_=tp[:cl, :],
                                         func=mybir.ActivationFunctionType.Copy)
                    if write_rows:
                        nc.sync.dma_start(out=hrows[cs:cs + cl, :], in_=rt[:cl, :])
                    if mm_readout:
                        nc.tensor.matmul(rps[:], rt[:cl, :], selb[:cl, w * G:(w + 1) * G],
                                         start=(w == 0), stop=(w == NW - 1))
                return rps

            # ===== layers =====
            layer_body(0)
            transpose_rows(True, False, None)
            nc.gpsimd.collective_compute(
                "AllGather", mybir.AluOpType.bypass,
                replica_groups=[list(range(NC))], ins=[hrows[:]], outs=[hfull[:]])
            layer_body(1)
            transpose_rows(True, False, None)
            nc.gpsimd.collective_compute(
                "AllGather", mybir.AluOpType.bypass,
                replica_groups=[list(range(NC))], ins=[hrows[:]], outs=[hfull[:]])
            layer_body(2)
            # ===== readout =====
            selb = cpool.tile([128, NW * G], bf16, tag="selb")
            nc.sync.dma_start(out=selb[:], in_=sel_d[:])
            rps = transpose_rows(False, True, selb)
            rsb = wpool.tile([128, G], f32, tag="rsb")
            nc.vector.tensor_copy(out=rsb[:], in_=rps[:])
            nc.sync.dma_start(out=ar_in[:], in_=rsb[:])
            nc.gpsimd.collective_compute(
                "AllReduce", mybir.AluOpType.add,
                replica_groups=[list(range(NC))], ins=[ar_in[:]], outs=[ar_out[:]])
            # ===== head =====
            wc1_t = cpool.tile([HID, HID], bf16, tag="wc1t")
            nc.sync.dma_start(out=wc1_t[:], in_=wc1_d[:])
            wc2_t = cpool.tile([HID, HID], bf16, tag="wc2t")
            nc.sync.dma_start(out=wc2_t[:], in_=wc2_d[:])
            wc3_t = cpool.tile([HID, 1], bf16, tag="wc3t")
            nc.sync.dma_start(out=wc3_t[:], in_=wc3_d[:])
            bc1_t = cpool.tile([HID, 1], f32, tag="bc1t")
            nc.sync.dma_start(out=bc1_t[:], in_=bc1_d[:])
            bc2_t = cpool.tile([HID, 1], f32, tag="bc2t")
            nc.sync.dma_start(out=bc2_t[:], in_=bc2_d[:])
            bc3_t = cpool.tile([1, 1], f32, tag="bc3t")
            nc.sync.dma_start(out=bc3_t[:], in_=bc3_d[:])
            rd = wpool.tile([128, G], f32, tag="rd")
            nc.sync.dma_start(out=rd[:], in_=ar_out[:])
            rdb = wpool.tile([128, G], bf16, tag="rdb")
            nc.vector.tensor_copy(out=rdb[:], in_=rd[:])
            h1p = pp.tile([128, G], f32, space="PSUM", tag="d")
            nc.tensor.matmul(h1p[:], wc1_t[:], rdb[:], start=True, stop=True)
            h1b = wpool.tile([128, G], bf16, tag="h1b")
            nc.scalar.activation(out=h1b[:], in_=h1p[:],
                                 func=mybir.ActivationFunctionType.Relu,
                                 bias=bc1_t[:], scale=1.0)
            h2p = pp.tile([128, G], f32, space="PSUM", tag="d")
            nc.tensor.matmul(h2p[:], wc2_t[:], h1b[:], start=True, stop=True)
            h2b = wpool.tile([128, G], bf16, tag="h2b")
            nc.scalar.activation(out=h2b[:], in_=h2p[:],
                                 func=mybir.ActivationFunctionType.Relu,
                                 bias=bc2_t[:], scale=1.0)
            op = pp.tile([1, G], f32, space="PSUM", tag="a")
            nc.tensor.matmul(op[:], wc3_t[:], h2b[:], start=True, stop=True)
            osb = wpool.tile([1, G], f32, tag="osb")
            nc.scalar.activation(out=osb[:], in_=op[:],
                                 func=mybir.ActivationFunctionType.Copy,
                                 bias=float(np.asarray(bc3).ravel()[0]), scale=1.0)
            nc.sync.dma_start(out=out_d[:], in_=osb[:])

    nc.finalize()

    in_maps = []
    W0n = np.asarray(W0, np.float32)
    Wln = np.asarray(Wl, np.float32)
    rootln = np.asarray(rootl, np.float32)
    shared = {
        "w0hi": np.ascontiguousarray(W0n[:, :128, :].transpose(1, 0, 2).reshape(128, R * HID)).astype(BF16),
        "w0lo": np.ascontiguousarray(W0n[:, 128:, :].transpose(1, 0, 2).reshape(IN - 128, R * HID)).astype(BF16),
        "wl": np.ascontiguousarray(Wln.transpose(2, 0, 1, 3).reshape(HID, L * R * HID)).astype(BF16),
        "root0": np.asarray(root0, np.float32).astype(BF16),
        "rootl": np.ascontiguousarray(rootln.transpose(1, 0, 2).reshape(HID, L * HID)).astype(BF16),
        "b0": np.asarray(b0, np.float32).reshape(HID, 1),
        "bl": np.ascontiguousarray(np.asarray(bl, np.float32).T),
        "wc1": np.asarray(Wc1, np.float32).astype(BF16),
        "wc2": np.asarray(Wc2, np.float32).astype(BF16),
        "wc3": np.asarray(Wc3, np.float32).astype(BF16),
        "bc1": np.asarray(bc1, np.float32).reshape(HID, 1),
        "bc2": np.asarray(bc2, np.float32).reshape(HID, 1),
        "bc3": np.asarray(bc3, np.float32).reshape(1, 1),
    }
    for c in range(NC):
        p = per_core[c]
        in_maps.append({
            "l0pack": p["l0pack"], "onehot": p["onehot"], "eidx": p["eidx"],
            "cidx": p["cidx"], "xmyt": p["xmyt"], "sel": p["sel"], **shared})
    import os, time as _time
    res = run_bass_kernel_spmd(nc, in_maps, list(range(NC)))
    if os.environ.get("RGCN_TIME") == "1":
        t0 = _time.time()
        res = run_bass_kernel_spmd(nc, in_maps, list(range(NC)))
        print("WARM_CALL_S:", _time.time() - t0)
    return np.asarray(res.results[0]["out"], np.float32).reshape(G, 1)



# revision 5
# speedup vs baseline: 10.5890x; 10.5890x over previous
"""RGCN (5 relations, 3 RGCN layers + mean readout + MLP head) on 8 trn2 cores.

Sharding: data-parallel over destination-node slices (12500/core). Per layer,
mean-aggregation of raw inputs per relation is done with one-hot matmuls on
the TensorEngine (aggregate-then-multiply), followed by one dense [f,h] matmul
per relation; h is kept feature-major (bf16) in SBUF. Cross-layer exchange is
an AllGather of row-major h. Gathers use dma_gather via per-relation compacted
tables (16384 rows, int16-safe) built on device. Readout via a host-built
selection-matrix matmul + AllReduce; the small MLP head is replicated.
"""

import sys
import numpy as np

sys.path.insert(0, "/opt/trn_rl_repo")

import ml_dtypes  # noqa: E402

BF16 = ml_dtypes.bfloat16

N = 100000
G = 256
E = 120000
IN = 162
HID = 128
R = 5
L = 2
NC = 8
SLICE = N // NC            # 12500
NW = (SLICE + 127) // 128  # 98 windows (last partial: 84 nodes)
TPW = 2                    # tiles per window (256-edge capacity per (r,w))
NTILES = NW * TPW          # 196
SLOTS = NTILES * 128       # 25088
QROWS = N // 4             # 25000
CCAP = 4096                # compact rows per quarter -> table 16384 rows
TROWS = 4 * CCAP
NCH = 7                    # stream chunks per relation
WPC = 7                    # window-pairs per chunk (NCH*WPC*2 == NW)
TPC = WPC * 2 * TPW        # 28 tiles per chunk
SPC = TPC * 128            # 3584 slots per chunk


def _wrap_idx(idx):
    n = len(idx)
    w = np.zeros((128, n // 16), np.int16)
    m = idx.reshape(n // 16, 16).T
    for k in range(8):
        w[16 * k:16 * (k + 1), :] = m
    return w


def _prep_core(c, X, srcs, dsts, batch_np, cnts):
    lo, hi = c * SLICE, (c + 1) * SLICE
    l0pack = np.zeros((R, 128, NTILES * IN), BF16)
    onehot = np.zeros((R, 128, NTILES * 128), BF16)
    eidx = np.zeros((R, 128, SLOTS // 16), np.int16)
    cidx = np.zeros((R, 4, 128, CCAP // 16), np.int16)
    for r in range(R):
        m = (dsts[r] >= lo) & (dsts[r] < hi)
        s = srcs[r][m]
        dg = dsts[r][m]
        o = np.argsort(dg - lo, kind="stable")
        s, dg = s[o], dg[o]
        d = dg - lo
        u = np.unique(s)
        uq = [u[(u >= q * QROWS) & (u < (q + 1) * QROWS)] for q in range(4)]
        for q in range(4):
            k = len(uq[q])
            assert k <= CCAP, (c, r, q, k)
            ci = np.zeros(CCAP, np.int16)
            ci[:k] = (uq[q] - q * QROWS).astype(np.int16)
            cidx[r, q] = _wrap_idx(ci)
        base = np.array([0, len(uq[0]), len(uq[0]) + len(uq[1]),
                         len(uq[0]) + len(uq[1]) + len(uq[2])])
        cat = np.concatenate(uq)
        qo = s // QROWS
        pos = (qo * CCAP + (np.searchsorted(cat, s) - base[qo])).astype(np.int64)
        w_of = d // 128
        wc = np.bincount(w_of, minlength=NW)
        assert wc.max() <= TPW * 128, (c, r, wc.max())
        start = np.concatenate([[0], np.cumsum(wc)[:-1]])
        slot = w_of * (TPW * 128) + (np.arange(len(d)) - start[w_of])
        e_arr = np.zeros(SLOTS, np.int64)
        e_arr[slot] = pos
        eidx[r] = _wrap_idx(e_arr.astype(np.int16))
        lane = slot % 128
        tcol = slot // 128
        colw = d - 128 * w_of
        vals = (1.0 / cnts[r][dg]).astype(np.float32)
        oh = np.zeros((128, NTILES, 128), np.float32)
        oh[lane, tcol, colw] = vals
        onehot[r] = oh.reshape(128, NTILES * 128).astype(BF16)
        lp = np.zeros((128, NTILES, IN), np.float32)
        lp[lane, tcol, :] = X[s]
        l0pack[r] = lp.reshape(128, NTILES * IN).astype(BF16)
    xmyt = np.ascontiguousarray(X[lo:hi].T).astype(BF16)
    sel = np.zeros((128, NW, G), np.float32)
    bat = batch_np[lo:hi]
    node = np.arange(SLICE)
    gcnt = np.maximum(np.bincount(batch_np, minlength=G), 1).astype(np.float32)
    sel[node % 128, node // 128, bat] = 1.0 / gcnt[bat]
    sel = sel.reshape(128, NW * G)
    return dict(l0pack=l0pack, onehot=onehot, eidx=eidx, cidx=cidx,
                xmyt=xmyt, sel=sel.astype(BF16))


def kernel(X, edge_index1, edge_index2, edge_index3, edge_index4, edge_index5,
           batch, W0, root0, b0, Wl, rootl, bl, Wc1, bc1, Wc2, bc2, Wc3, bc3):
    import os as _os
    import time as _t
    _dbg = _os.environ.get("RGCN_DEBUG") == "1"
    _tprev = [_t.time()]

    def _mark(tag):
        if _dbg:
            now = _t.time()
            print(f"[rgcn-timing] {tag}: {now - _tprev[0]:.3f}s", flush=True)
            _tprev[0] = now

    import concourse.bass as bass  # noqa: F401
    import concourse.bacc as bacc
    import concourse.mybir as mybir
    import concourse.tile as tile
    from concourse.bass_utils import run_bass_kernel_spmd
    from concourse.masks import make_identity
    _mark("imports")

    X = np.asarray(X, np.float32)
    batch_np = np.asarray(batch).astype(np.int64)
    eis = [np.asarray(e).astype(np.int64) for e in
           (edge_index1, edge_index2, edge_index3, edge_index4, edge_index5)]
    srcs = [e[0] for e in eis]
    dsts = [e[1] for e in eis]
    cnts = [np.maximum(np.bincount(dsts[r], minlength=N), 1).astype(np.float32)
            for r in range(R)]
    per_core = [_prep_core(c, X, srcs, dsts, batch_np, cnts) for c in range(NC)]
    _mark("host prep")

    f32, bf16, i16 = mybir.dt.float32, mybir.dt.bfloat16, mybir.dt.int16
    f16 = mybir.dt.float16
    f8 = mybir.dt.float8e4

    nc = bacc.Bacc("TRN2", target_bir_lowering=False, debug=False)
    l0pack_d = nc.declare_dram_parameter("l0pack", [R, 128, NTILES * IN], bf16, isOutput=False)
    onehot_d = nc.declare_dram_parameter("onehot", [R, 128, NTILES * 128], bf16, isOutput=False)
    eidx_d = nc.declare_dram_parameter("eidx", [R, 128, SLOTS // 16], i16, isOutput=False)
    cidx_d = nc.declare_dram_parameter("cidx", [R, 4, 128, CCAP // 16], i16, isOutput=False)
    xmyt_d = nc.declare_dram_parameter("xmyt", [IN, SLICE], bf16, isOutput=False)
    sel_d = nc.declare_dram_parameter("sel", [128, NW * G], bf16, isOutput=False)
    w0hi_d = nc.declare_dram_parameter("w0hi", [128, R * HID], bf16, isOutput=False)
    w0lo_d = nc.declare_dram_parameter("w0lo", [IN - 128, R * HID], bf16, isOutput=False)
    wl_d = nc.declare_dram_parameter("wl", [HID, L * R * HID], bf16, isOutput=False)
    root0_d = nc.declare_dram_parameter("root0", [IN, HID], bf16, isOutput=False)
    rootl_d = nc.declare_dram_parameter("rootl", [HID, L * HID], bf16, isOutput=False)
    b0_d = nc.declare_dram_parameter("b0", [HID, 1], f32, isOutput=False)
    bl_d = nc.declare_dram_parameter("bl", [HID, L], f32, isOutput=False)
    wc1_d = nc.declare_dram_parameter("wc1", [HID, HID], bf16, isOutput=False)
    wc2_d = nc.declare_dram_parameter("wc2", [HID, HID], bf16, isOutput=False)
    wc3_d = nc.declare_dram_parameter("wc3", [HID, 1], bf16, isOutput=False)
    bc1_d = nc.declare_dram_parameter("bc1", [HID, 1], f32, isOutput=False)
    bc2_d = nc.declare_dram_parameter("bc2", [HID, 1], f32, isOutput=False)
    bc3_d = nc.declare_dram_parameter("bc3", [1, 1], f32, isOutput=False)
    out_d = nc.declare_dram_parameter("out", [1, G], f32, isOutput=True)

    hrows = nc.dram_tensor("hrows", [SLICE, HID], bf16)
    hfull = nc.dram_tensor("hfull", [N, HID], bf16, addr_space="Shared")
    trs = [nc.dram_tensor(f"tr{r}", [TROWS, HID], bf16) for r in range(R)]
    ar_in = nc.dram_tensor("ar_in", [HID, G], f32)
    ar_out = nc.dram_tensor("ar_out", [HID, G], f32, addr_space="Shared")

    WPAIR = NW // 2

    with tile.TileContext(nc) as tc:
        with tc.tile_pool(name="const", bufs=1) as cpool, \
             tc.tile_pool(name="hbuf", bufs=1) as hpool, \
             tc.tile_pool(name="work", bufs=3) as wpool, \
             tc.tile_pool(name="stg", bufs=2) as spool, \
             tc.tile_pool(name="edge", bufs=3) as epool, \
             tc.tile_pool(name="oh", bufs=3) as opool, \
             tc.tile_pool(name="ps", bufs=2, space="PSUM") as pp:

            ident = cpool.tile([128, 128], bf16, tag="ident")
            make_identity(nc, ident[:])
            w0hi_t = cpool.tile([128, R * HID], bf16, tag="w0hi")
            nc.sync.dma_start(out=w0hi_t[:], in_=w0hi_d[:])
            w0lo_t = cpool.tile([IN - 128, R * HID], bf16, tag="w0lo")
            nc.sync.dma_start(out=w0lo_t[:], in_=w0lo_d[:])
            wl_t = cpool.tile([HID, L * R * HID], bf16, tag="wlt")
            nc.sync.dma_start(out=wl_t[:], in_=wl_d[:])
            root0hi_t = cpool.tile([128, HID], bf16, tag="root0hi")
            nc.sync.dma_start(out=root0hi_t[:], in_=root0_d[0:128, :])
            root0lo_t = cpool.tile([IN - 128, HID], bf16, tag="root0lo")
            nc.sync.dma_start(out=root0lo_t[:], in_=root0_d[128:IN, :])
            rootl_t = cpool.tile([HID, L * HID], bf16, tag="rootlt")
            nc.sync.dma_start(out=rootl_t[:], in_=rootl_d[:])
            b0_t = cpool.tile([HID, 1], f32, tag="b0t")
            nc.sync.dma_start(out=b0_t[:], in_=b0_d[:])
            bl_t = cpool.tile([HID, L], f32, tag="blt")
            nc.sync.dma_start(out=bl_t[:], in_=bl_d[:])

            h_cur = hpool.tile([128, SLICE], bf16, tag="hcur")
            h_acc = hpool.tile([128, SLICE], f16, tag="hacc")

            def layer_body(layer):
                fstep = IN if layer == 0 else HID
                # --- init h_acc with root contribution (reads old h_cur) ---
                for wp in range(WPAIR):
                    cs = wp * 256
                    cl = min(256, SLICE - cs)
                    ps = pp.tile([128, 256], f32, space="PSUM", tag="d")
                    if layer == 0:
                        xh = wpool.tile([128, 256], bf16, tag="xsl")
                        nc.sync.dma_start(out=xh[:, :cl], in_=xmyt_d[0:128, cs:cs + cl])
                        xl = wpool.tile([IN - 128, 256], bf16, tag="xsl2")
                        nc.sync.dma_start(out=xl[:, :cl], in_=xmyt_d[128:IN, cs:cs + cl])
                        nc.tensor.matmul(ps[:, :cl], root0hi_t[:], xh[:, :cl], start=True, stop=False)
                        nc.tensor.matmul(ps[:, :cl], root0lo_t[:], xl[:, :cl], start=False, stop=True)
                    else:
                        nc.tensor.matmul(ps[:, :cl], rootl_t[:, (layer - 1) * HID:layer * HID], h_cur[:, cs:cs + cl], start=True, stop=True)
                    nc.scalar.activation(out=h_acc[:, cs:cs + cl], in_=ps[:, :cl],
                                         func=mybir.ActivationFunctionType.Copy)
                for r in range(R):
                    wmat = None if layer == 0 else wl_t[:, ((layer - 1) * R + r) * HID:((layer - 1) * R + r + 1) * HID]
                    if layer != 0:
                        for q in range(4):
                            ci = wpool.tile([128, CCAP // 16], i16, tag="ci")
                            nc.sync.dma_start(out=ci[:], in_=cidx_d[r, q])
                            st = spool.tile([128, (CCAP // 128) * HID], bf16, tag="stage")
                            nc.gpsimd.dma_gather(
                                out_ap=st[:].rearrange("p (t f) -> p t f", f=HID),
                                in_ap=hfull[q * QROWS:(q + 1) * QROWS, :],
                                idxs_ap=ci[:],
                                num_idxs=CCAP, num_idxs_reg=CCAP,
                                elem_size=HID, single_packet=False)
                            nc.sync.dma_start(
                                out=trs[r][q * CCAP:(q + 1) * CCAP, :].rearrange(
                                    "(t p) f -> p t f", p=128),
                                in_=st[:].rearrange("p (t f) -> p t f", f=HID))
                        ei = spool.tile([128, SLOTS // 16], i16, tag="ei")
                        nc.sync.dma_start(out=ei[:], in_=eidx_d[r])
                    for ch in range(NCH):
                        t0 = ch * TPC
                        ebuf = epool.tile([128, TPC * IN], bf16, tag="ebuf")
                        if layer == 0:
                            nc.sync.dma_start(
                                out=ebuf[:, :TPC * IN],
                                in_=l0pack_d[r][:, t0 * IN:(t0 + TPC) * IN])
                        else:
                            nc.gpsimd.dma_gather(
                                out_ap=ebuf[:, :TPC * HID].rearrange("p (t f) -> p t f", f=HID),
                                in_ap=trs[r][:],
                                idxs_ap=ei[:, (t0 * 128) // 16:((t0 + TPC) * 128) // 16],
                                num_idxs=SPC, num_idxs_reg=SPC,
                                elem_size=HID, single_packet=False)
                        ohb = opool.tile([128, TPC * 128], bf16, tag="ohb")
                        nc.sync.dma_start(
                            out=ohb[:], in_=onehot_d[r][:, t0 * 128:(t0 + TPC) * 128])
                        for wpl in range(WPC):
                            wp = ch * WPC + wpl
                            aps = pp.tile([128, 256], f32, space="PSUM", tag="a")
                            if layer == 0:
                                aps2 = pp.tile([IN - 128, 256], f32, space="PSUM", tag="a2")
                            for half in range(2):
                                for t in range(TPW):
                                    ti = (wpl * 2 + half) * TPW + t
                                    et = ebuf[:, ti * fstep:ti * fstep + fstep]
                                    oh = ohb[:, ti * 128:(ti + 1) * 128]
                                    st0, sp0 = (t == 0), (t == TPW - 1)
                                    nc.tensor.matmul(
                                        aps[:, half * 128:(half + 1) * 128],
                                        et[:, 0:128], oh, start=st0, stop=sp0)
                                    if layer == 0:
                                        nc.tensor.matmul(
                                            aps2[:, half * 128:(half + 1) * 128],
                                            et[:, 128:IN], oh, start=st0, stop=sp0)
                            a_sb = wpool.tile([128, 256], bf16, tag="asb")
                            nc.scalar.activation(out=a_sb[:], in_=aps[:],
                                                 func=mybir.ActivationFunctionType.Copy)
                            dps = pp.tile([128, 256], f32, space="PSUM", tag="d")
                            if layer == 0:
                                a_sb2 = wpool.tile([IN - 128, 256], bf16, tag="asb2")
                                nc.scalar.activation(out=a_sb2[:], in_=aps2[:],
                                                     func=mybir.ActivationFunctionType.Copy)
                                nc.tensor.matmul(dps[:], w0hi_t[:, r * HID:(r + 1) * HID], a_sb[:], start=True, stop=False)
                                nc.tensor.matmul(dps[:], w0lo_t[:, r * HID:(r + 1) * HID], a_sb2[:], start=False, stop=True)
                            else:
                                nc.tensor.matmul(dps[:], wmat, a_sb[:], start=True, stop=True)
                            cs = wp * 256
                            cl = min(256, SLICE - cs)
                            nc.vector.tensor_tensor(
                                out=h_acc[:, cs:cs + cl], in0=dps[:, :cl],
                                in1=h_acc[:, cs:cs + cl], op=mybir.AluOpType.add)
                bias = b0_t[:] if layer == 0 else bl_t[:, layer - 1:layer]
                for wp in range(WPAIR):
                    cs = wp * 256
                    cl = min(256, SLICE - cs)
                    nc.scalar.activation(
                        out=h_cur[:, cs:cs + cl], in_=h_acc[:, cs:cs + cl],
                        func=mybir.ActivationFunctionType.Relu,
                        bias=bias, scale=1.0)

            def transpose_rows(write_rows, mm_readout, selb):
                rps = None
                if mm_readout:
                    rps = pp.tile([128, G], f32, space="PSUM", tag="a2")
                for w in range(NW):
                    cs = w * 128
                    cl = min(128, SLICE - cs)
                    tp = pp.tile([128, 128], bf16, space="PSUM", tag="a")
                    nc.tensor.transpose(out=tp[:cl, :], in_=h_cur[:, cs:cs + cl], identity=ident[:])
                    rt = wpool.tile([128, 128], bf16, tag="rowt")
                    nc.scalar.activation(out=rt[:cl, :], in_=tp[:cl, :],
                                         func=mybir.ActivationFunctionType.Copy)
                    if write_rows:
                        nc.sync.dma_start(out=hrows[cs:cs + cl, :], in_=rt[:cl, :])
                    if mm_readout:
                        nc.tensor.matmul(rps[:], rt[:cl, :], selb[:cl, w * G:(w + 1) * G],
                                         start=(w == 0), stop=(w == NW - 1))
                return rps

            # ===== layers =====
            layer_body(0)
            transpose_rows(True, False, None)
            nc.gpsimd.collective_compute(
                "AllGather", mybir.AluOpType.bypass,
                replica_groups=[list(range(NC))], ins=[hrows[:]], outs=[hfull[:]])
            layer_body(1)
            transpose_rows(True, False, None)
            nc.gpsimd.collective_compute(
                "AllGather", mybir.AluOpType.bypass,
                replica_groups=[list(range(NC))], ins=[hrows[:]], outs=[hfull[:]])
            layer_body(2)
            # ===== readout =====
            selb = cpool.tile([128, NW * G], bf16, tag="selb")
            nc.sync.dma_start(out=selb[:], in_=sel_d[:])
            rps = transpose_rows(False, True, selb)
            rsb = wpool.tile([128, G], f32, tag="rsb")
            nc.vector.tensor_copy(out=rsb[:], in_=rps[:])
            nc.sync.dma_start(out=ar_in[:], in_=rsb[:])
            nc.gpsimd.collective_compute(
                "AllReduce", mybir.AluOpType.add,
                replica_groups=[list(range(NC))], ins=[ar_in[:]], outs=[ar_out[:]])
            # ===== head =====
            wc1_t = cpool.tile([HID, HID], bf16, tag="wc1t")
            nc.sync.dma_start(out=wc1_t[:], in_=wc1_d[:])
            wc2_t = cpool.tile([HID, HID], bf16, tag="wc2t")
            nc.sync.dma_start(out=wc2_t[:], in_=wc2_d[:])
            wc3_t = cpool.tile([HID, 1], bf16, tag="wc3t")
            nc.sync.dma_start(out=wc3_t[:], in_=wc3_d[:])
            bc1_t = cpool.tile([HID, 1], f32, tag="bc1t")
            nc.sync.dma_start(out=bc1_t[:], in_=bc1_d[:])
            bc2_t = cpool.tile([HID, 1], f32, tag="bc2t")
            nc.sync.dma_start(out=bc2_t[:], in_=bc2_d[:])
            bc3_t = cpool.tile([1, 1], f32, tag="bc3t")
            nc.sync.dma_start(out=bc3_t[:], in_=bc3_d[:])
            rd = wpool.tile([128, G], f32, tag="rd")
            nc.sync.dma_start(out=rd[:], in_=ar_out[:])
            rdb = wpool.tile([128, G], bf16, tag="rdb")
            nc.vector.tensor_copy(out=rdb[:], in_=rd[:])
            h1p = pp.tile([128, G], f32, space="PSUM", tag="d")
            nc.tensor.matmul(h1p[:], wc1_t[:], rdb[:], start=True, stop=True)
            h1b = wpool.tile([128, G], bf16, tag="h1b")
            nc.scalar.activation(out=h1b[:], in_=h1p[:],
                                 func=mybir.ActivationFunctionType.Relu,
                                 bias=bc1_t[:], scale=1.0)
            h2p = pp.tile([128, G], f32, space="PSUM", tag="d")
            nc.tensor.matmul(h2p[:], wc2_t[:], h1b[:], start=True, stop=True)
            h2b = wpool.tile([128, G], bf16, tag="h2b")
            nc.scalar.activation(out=h2b[:], in_=h2p[:],
                                 func=mybir.ActivationFunctionType.Relu,
                                 bias=bc2_t[:], scale=1.0)
            op = pp.tile([1, G], f32, space="PSUM", tag="a")
            nc.tensor.matmul(op[:], wc3_t[:], h2b[:], start=True, stop=True)
            osb = wpool.tile([1, G], f32, tag="osb")
            nc.scalar.activation(out=osb[:], in_=op[:],
                                 func=mybir.ActivationFunctionType.Copy,
                                 bias=float(np.asarray(bc3).ravel()[0]), scale=1.0)
            nc.sync.dma_start(out=out_d[:], in_=osb[:])

    _mark("trace")
    nc.finalize()
    _mark("finalize")

    in_maps = []
    W0n = np.asarray(W0, np.float32)
    Wln = np.asarray(Wl, np.float32)
    rootln = np.asarray(rootl, np.float32)
    shared = {
        "w0hi": np.ascontiguousarray(W0n[:, :128, :].transpose(1, 0, 2).reshape(128, R * HID)).astype(BF16),
        "w0lo": np.ascontiguousarray(W0n[:, 128:, :].transpose(1, 0, 2).reshape(IN - 128, R * HID)).astype(BF16),
        "wl": np.ascontiguousarray(Wln.transpose(2, 0, 1, 3).reshape(HID, L * R * HID)).astype(BF16),
        "root0": np.asarray(root0, np.float32).astype(BF16),
        "rootl": np.ascontiguousarray(rootln.transpose(1, 0, 2).reshape(HID, L * HID)).astype(BF16),
        "b0": np.asarray(b0, np.float32).reshape(HID, 1),
        "bl": np.ascontiguousarray(np.asarray(bl, np.float32).T),
        "wc1": np.asarray(Wc1, np.float32).astype(BF16),
        "wc2": np.asarray(Wc2, np.float32).astype(BF16),
        "wc3": np.asarray(Wc3, np.float32).astype(BF16),
        "bc1": np.asarray(bc1, np.float32).reshape(HID, 1),
        "bc2": np.asarray(bc2, np.float32).reshape(HID, 1),
        "bc3": np.asarray(bc3, np.float32).reshape(1, 1),
    }
    for c in range(NC):
        p = per_core[c]
        in_maps.append({
            "l0pack": p["l0pack"], "onehot": p["onehot"], "eidx": p["eidx"],
            "cidx": p["cidx"], "xmyt": p["xmyt"], "sel": p["sel"], **shared})
    import os, time as _time
    _mark("in_maps")
    res = run_bass_kernel_spmd(nc, in_maps, list(range(NC)))
    _mark("run (compile+exec)")
    if os.environ.get("RGCN_TIME") == "1":
        t0 = _time.time()
        res = run_bass_kernel_spmd(nc, in_maps, list(range(NC)))
        print("WARM_CALL_S:", _time.time() - t0)
    return np.asarray(res.results[0]["out"], np.float32).reshape(G, 1)

